# revision 1
# baseline (speedup 1.0000x reference)
"""Trainium2 Bass kernel for the Gaussian-span multi-head self-attention module.

  span  = head_reshape(h @ W_span.T, 2)          (B*K, M, 2)
  value = head_reshape(h @ W_val.T, D)           (B*K, M, D)
  mean  = sigmoid(span0) * M ; soft = softplus(span1)
  attn  = softmax(-soft * (pos - mean)^2)        (B*K, M, M)
  out   = (attn @ value)  -> concat heads -> @ W_out.T

Shapes are hardcoded: B=2, M=2048, HS=1024, K=16 heads, D=64.

Strategy (8 NeuronCores, SPMD — one program, per-core data):
  * batch*head sharding: core = b*4 + g handles batch b, heads [4g, 4g+4).
  * The Gaussian attention rows are extremely localized (soft >= ~0.01 means the
    window exp(-soft*(j-mean)^2) dies within |j-mean| <~ sqrt(50/soft)).  The host
    computes mean/soft (the tiny span projection, 0.25% of total FLOPs), sorts each
    head's query rows by mean, and builds a windowed schedule: for each 128-wide
    key block jb, only the contiguous range of sorted rows whose window touches it
    is processed (~9x fewer score elements than dense).  The schedule is the
    envelope over all 32 head-instances, so all 8 cores share one NEFF.
  * Scores are produced on the TensorEngine as a rank-3 matmul (basis [u^2, u, 1]
    centered per key block x host-precomputed coefficients [s, -2 s t, s t^2]),
    then a single ScalarEngine exp(-x) pass yields attention weights in bf16.
  * attn @ value accumulates out^T (65 x M) in PSUM with the value tile as the
    stationary operand; a ones-column in the value tile produces the softmax
    denominator for free.  Normalization uses a K=1 broadcast matmul + DVE mult.
  * The sorted->natural row un-permutation runs on the otherwise-idle GPSIMD
    engine via local_scatter on (head-pair x M) bf16 tiles.
  * Output projection is a bf16 matmul per 128-row block; per-core partials
    (one per batch half) are summed on the host.
"""

import sys
import types

import numpy as np
import ml_dtypes

B, M, HS, NH, D = 2, 2048, 1024, 16, 64
NCORES = 8
HPC = 4            # heads per core
CP = HPC * D       # 256-wide channel slice per core
SUB = 512          # scores i sub-chunk
NJB = M // 128     # key blocks
TAIL_T = 50.0      # window cut: exp(-TAIL_T) ~ 2e-22

_CACHE = {}        # ranges tuple -> compiled Bass program


def _ensure_ntff_hook():
    """Install the antenv.axon_hooks shim if the image lacks it (profiling only)."""
    try:
        import antenv.axon_hooks  # noqa: F401
        return
    except ImportError:
        pass
    try:
        import antenv
        from trn_agent_boot.trn_boot import _ntff_profile_via_ctypes
    except ImportError:
        return
    mod = types.ModuleType("antenv.axon_hooks")
    _h = [None]
    mod.set_axon_ntff_profile_hook = lambda hk: _h.__setitem__(0, hk)
    mod.get_axon_ntff_profile_hook = lambda: _h[0]
    sys.modules["antenv.axon_hooks"] = mod
    antenv.axon_hooks = mod
    try:
        mod.set_axon_ntff_profile_hook(
            _ntff_profile_via_ctypes("/opt/axon/libaxon_pjrt.so"))
    except Exception:
        pass


def _sigmoid64(x):
    return 1.0 / (1.0 + np.exp(-x.astype(np.float64)))


def _softplus64(x):
    return np.logaddexp(0.0, x.astype(np.float64))


def _build_host_data(h, W_span, W_val, W_out):
    h = np.asarray(h, np.float32)
    W_span = np.asarray(W_span, np.float32)
    W_val = np.asarray(W_val, np.float32)
    W_out = np.asarray(W_out, np.float32)

    span = (h.reshape(B * M, HS) @ W_span.T).reshape(B, M, 2 * NH)

    m_all = np.zeros((B, NH, M), np.float64)
    s_all = np.zeros((B, NH, M), np.float64)
    for b in range(B):
        for k in range(NH):
            m_all[b, k] = _sigmoid64(span[b, :, 2 * k]) * M
            s_all[b, k] = _softplus64(span[b, :, 2 * k + 1])
    order_all = np.argsort(m_all, axis=-1, kind="stable")
    W_all = np.sqrt(TAIL_T / np.maximum(s_all, 1e-12))

    ilos = np.full(NJB, M, np.int64)
    ihis = np.zeros(NJB, np.int64)
    for b in range(B):
        for k in range(NH):
            ms = m_all[b, k][order_all[b, k]]
            ws = W_all[b, k][order_all[b, k]]
            lo, hi = ms - ws, ms + ws
            for jb in range(NJB):
                mask = (hi >= jb * 128) & (lo <= jb * 128 + 128)
                idx = np.flatnonzero(mask)
                if idx.size:
                    ilos[jb] = min(ilos[jb], idx[0])
                    ihis[jb] = max(ihis[jb], idx[-1] + 1)
    ranges = []
    for jb in range(NJB):
        if ihis[jb] <= ilos[jb]:
            ranges.append((0, 0))
        else:
            ranges.append((int(ilos[jb]) & ~7, min(M, (int(ihis[jb]) + 7) & ~7)))

    # coverage: every sorted row must fall in the range of its own mean's block
    for b in range(B):
        for k in range(NH):
            ms = m_all[b, k][order_all[b, k]]
            own = np.clip((ms // 128).astype(np.int64), 0, NJB - 1)
            pos = np.arange(M)
            lows = np.array([ranges[j][0] for j in own])
            highs = np.array([ranges[j][1] for j in own])
            if not ((lows <= pos) & (pos < highs)).all():
                raise AssertionError("window schedule does not cover all rows")

    maxw = max(max(hi - lo for lo, hi in ranges), 8)

    in_maps = []
    for core in range(NCORES):
        b, g = core // HPC, core % HPC
        heads = [g * HPC + kk for kk in range(HPC)]

        hTb = np.ascontiguousarray(h[b].T).astype(ml_dtypes.bfloat16)
        Wv = np.ascontiguousarray(W_val[g * CP:(g + 1) * CP, :].T).astype(ml_dtypes.bfloat16)
        Wo = np.ascontiguousarray(W_out[:, g * CP:(g + 1) * CP].T).astype(ml_dtypes.bfloat16)

        A3 = np.zeros((HPC, 3, NJB, maxw), np.float32)
        sidx = np.zeros((2, 2, 128, M), np.int16)
        for kk, k in enumerate(heads):
            order = order_all[b, k]
            ms = m_all[b, k][order]
            ss = s_all[b, k][order]
            for jb in range(NJB):
                lo, hi = ranges[jb]
                if hi <= lo:
                    continue
                t = ms[lo:hi] - (128.0 * jb + 64.0)
                s_ = ss[lo:hi]
                A3[kk, 0, jb, :hi - lo] = s_
                A3[kk, 1, jb, :hi - lo] = -2.0 * s_ * t
                A3[kk, 2, jb, :hi - lo] = s_ * t * t
            pair, sub = kk // 2, kk % 2
            o64 = order.astype(np.int64)
            iA = np.where(o64 < M // 2, o64, -1).astype(np.int16)
            iB = np.where(o64 >= M // 2, o64 - M // 2, -1).astype(np.int16)
            rows = slice(64 * sub, 64 * sub + 64)
            sidx[pair, 0, rows, :] = iA[None, :]
            sidx[pair, 1, rows, :] = iB[None, :]

        u = np.arange(-64, 64, dtype=np.float32)
        basis = np.stack([u * u, u, np.ones(128, np.float32)])

        in_maps.append({
            "hTb": hTb, "Wv": Wv, "Wo": Wo,
            "A3": A3.reshape(HPC, 3, NJB * maxw),
            "sidx": sidx, "basis": basis,
        })

    return in_maps, tuple(ranges)


def _build_kernel(ranges):
    import concourse.tile as tile
    from concourse import bacc, mybir
    from concourse.alu_op_type import AluOpType

    F32 = mybir.dt.float32
    BF16 = mybir.dt.bfloat16
    I16 = mybir.dt.int16

    nc = bacc.Bacc("TRN2", target_bir_lowering=False, debug=False, num_devices=NCORES)

    maxw = max(max(hi - lo for lo, hi in ranges), 8)
    hTb = nc.dram_tensor("hTb", [HS, M], BF16, kind="ExternalInput")
    Wv = nc.dram_tensor("Wv", [HS, CP], BF16, kind="ExternalInput")
    Wo = nc.dram_tensor("Wo", [CP, HS], BF16, kind="ExternalInput")
    A3 = nc.dram_tensor("A3", [HPC, 3, NJB * maxw], F32, kind="ExternalInput")
    sidx = nc.dram_tensor("sidx", [2, 2, 128, M], I16, kind="ExternalInput")
    basis = nc.dram_tensor("basis", [3, 128], F32, kind="ExternalInput")
    out_part = nc.dram_tensor("out_part", [M, HS], BF16, kind="ExternalOutput")

    NC8 = HS // 128
    copy_ctr = [0]

    def copy_cast(out_ap, in_ap):
        if copy_ctr[0] % 2 == 0:
            nc.vector.tensor_copy(out_ap, in_ap)
        else:
            nc.scalar.copy(out_ap, in_ap)
        copy_ctr[0] += 1

    with tile.TileContext(nc) as tc:
        with (
            tc.tile_pool(name="persist", bufs=1) as persist,
            tc.tile_pool(name="vpool", bufs=1) as vpool,
            tc.tile_pool(name="attn_pool", bufs=3) as attn_pool,
            tc.tile_pool(name="norm_pool", bufs=4) as norm_pool,
            tc.tile_pool(name="out_pool", bufs=3) as out_pool,
            tc.tile_pool(name="ps", bufs=2, space="PSUM") as ps,
        ):
            hT_sb = []
            for c in range(NC8):
                t = persist.tile([128, M], BF16, name=f"hT{c}")
                nc.sync.dma_start(t[:], hTb[c * 128:(c + 1) * 128, :])
                hT_sb.append(t)
            Wv_sb = []
            for c in range(NC8):
                t = persist.tile([128, CP], BF16, name=f"Wv{c}")
                nc.sync.dma_start(t[:], Wv[c * 128:(c + 1) * 128, :])
                Wv_sb.append(t)
            Wo_sb = []
            for p in range(2):
                t = persist.tile([128, HS], BF16, name=f"Wo{p}")
                nc.sync.dma_start(t[:], Wo[p * 128:(p + 1) * 128, :])
                Wo_sb.append(t)
            A_sb = []
            for kk in range(HPC):
                t = persist.tile([3, NJB * maxw], F32, name=f"A{kk}")
                nc.sync.dma_start(t[:], A3[kk])
                A_sb.append(t)
            sidx_sb = []
            for p in range(2):
                row = []
                for hh in range(2):
                    t = persist.tile([128, M], I16, name=f"sidx{p}{hh}")
                    nc.sync.dma_start(t[:], sidx[p, hh])
                    row.append(t)
                sidx_sb.append(row)
            basis_sb = persist.tile([3, 128], F32, name="basis")
            nc.sync.dma_start(basis_sb[:], basis[:])
            ones_sb = persist.tile([1, 64], F32, name="ones64")
            nc.vector.memset(ones_sb[:], 1.0)

            pair_sb = [persist.tile([128, M], BF16, name=f"pair{p}") for p in range(2)]
            nat_sb = [persist.tile([128, M], BF16, name=f"nat{p}") for p in range(2)]

            # ---- value (per key block), with a ones column per head ----
            v_sb = []
            for jb in range(NJB):
                vt = vpool.tile([128, HPC * (D + 1)], BF16, name=f"v{jb}")
                pv = ps.tile([128, CP], F32, name="pv", tag="sc", bufs=2)
                for c in range(NC8):
                    nc.tensor.matmul(
                        pv[:], hT_sb[c][:, jb * 128:(jb + 1) * 128], Wv_sb[c][:],
                        start=(c == 0), stop=(c == NC8 - 1))
                for kk in range(HPC):
                    copy_cast(vt[:, kk * (D + 1):kk * (D + 1) + D],
                              pv[:, kk * D:(kk + 1) * D])
                    nc.vector.memset(vt[:, kk * (D + 1) + D:kk * (D + 1) + D + 1], 1.0)
                v_sb.append(vt)

            # ---- scores + attention per head ----
            for kk in range(HPC):
                pair, sub = kk // 2, kk % 2
                o_ps = ps.tile([65, M], F32, name="oT", tag="oT", bufs=1)
                nc.scalar.memzero(o_ps[:])
                for jb in range(NJB):
                    lo, hi = ranges[jb]
                    if hi <= lo:
                        continue
                    for s0 in range(lo, hi, SUB):
                        s1 = min(hi, s0 + SUB)
                        w = s1 - s0
                        sc = ps.tile([128, SUB], F32, name="sc", tag="sc", bufs=2)
                        nc.tensor.matmul(
                            sc[:, :w], basis_sb[:],
                            A_sb[kk][:, jb * maxw + s0 - lo: jb * maxw + s1 - lo],
                            start=True, stop=True)
                        at = attn_pool.tile([128, SUB], BF16, name="at", tag="at")
                        nc.scalar.activation(at[:, :w], sc[:, :w],
                                             mybir.ActivationFunctionType.Exp, scale=-1.0)
                        nc.tensor.matmul(
                            o_ps[:, s0:s1],
                            v_sb[jb][:, kk * (D + 1):(kk + 1) * (D + 1)],
                            at[:, :w], start=False, stop=False)
                for q in range(M // SUB):
                    qs = slice(q * SUB, (q + 1) * SUB)
                    rc = norm_pool.tile([1, SUB], F32, name="rc", tag="rc")
                    nc.vector.reciprocal(rc[:], o_ps[64:65, qs])
                    bc = ps.tile([64, SUB], F32, name="bc", tag="sc", bufs=2)
                    nc.tensor.matmul(bc[:], ones_sb[:], rc[:], start=True, stop=True)
                    bcs = norm_pool.tile([64, SUB], F32, name="bcs", tag="bcs")
                    nc.scalar.copy(bcs[:], bc[:])
                    nc.vector.tensor_tensor(
                        pair_sb[pair][64 * sub:64 * sub + 64, qs],
                        o_ps[0:64, qs], bcs[:], AluOpType.mult)

            # ---- un-permute sorted -> natural (gpsimd) ----
            for p in range(2):
                for hh in range(2):
                    nc.gpsimd.local_scatter(
                        nat_sb[p][:, hh * (M // 2):(hh + 1) * (M // 2)],
                        pair_sb[p][:], sidx_sb[p][hh][:],
                        channels=128, num_elems=M // 2, num_idxs=M)

            # ---- output projection ----
            for ic in range(M // 128):
                ics = slice(ic * 128, (ic + 1) * 128)
                ot = out_pool.tile([128, HS], BF16, name="ot", tag="ot")
                for jh in range(2):
                    jhs = slice(jh * 512, (jh + 1) * 512)
                    pp = ps.tile([128, 512], F32, name="pp", tag="pp", bufs=2)
                    nc.tensor.matmul(pp[:], nat_sb[0][:, ics], Wo_sb[0][:, jhs],
                                     start=True, stop=False)
                    nc.tensor.matmul(pp[:], nat_sb[1][:, ics], Wo_sb[1][:, jhs],
                                     start=False, stop=True)
                    copy_cast(ot[:, jhs], pp[:])
                nc.sync.dma_start(out_part[ics, :], ot[:])

    nc.compile()
    return nc


def kernel(h, W_span, W_val, W_out):
    _ensure_ntff_hook()
    from concourse.bass_utils import run_bass_kernel_spmd

    in_maps, ranges = _build_host_data(h, W_span, W_val, W_out)
    nc = _CACHE.get(ranges)
    if nc is None:
        nc = _build_kernel(ranges)
        _CACHE[ranges] = nc

    res = run_bass_kernel_spmd(nc, in_maps, list(range(NCORES)), trace=False)

    out = np.zeros((B, M, HS), np.float32)
    for core in range(NCORES):
        out[core // HPC] += res.results[core]["out_part"].astype(np.float32)
    return out


# revision 4
# speedup vs baseline: 1.1185x; 1.1185x over previous
"""Trainium2 Bass kernel for the Gaussian-span multi-head self-attention module.

  span  = head_reshape(h @ W_span.T, 2)          (B*K, M, 2)
  value = head_reshape(h @ W_val.T, D)           (B*K, M, D)
  mean  = sigmoid(span0) * M ; soft = softplus(span1)
  attn  = softmax(-soft * (pos - mean)^2)        (B*K, M, M)
  out   = (attn @ value)  -> concat heads -> @ W_out.T

Shapes are hardcoded: B=2, M=2048, HS=1024, K=16 heads, D=64.

Strategy (8 NeuronCores, SPMD — one program, per-core data):
  * batch*head sharding: core = b*4 + g handles batch b, heads [4g, 4g+4).
  * The Gaussian attention rows are extremely localized (soft >= ~0.01 means the
    window exp(-soft*(j-mean)^2) dies within |j-mean| <~ sqrt(50/soft)).  The host
    computes mean/soft (the tiny span projection, 0.25% of total FLOPs), sorts each
    head's query rows by mean, and builds a windowed schedule: for each 128-wide
    key block jb, only the contiguous range of sorted rows whose window touches it
    is processed (~9x fewer score elements than dense).  The schedule is the
    envelope over all 32 head-instances, so all 8 cores share one NEFF.
  * Scores are produced on the TensorEngine as a rank-3 matmul (basis [u^2, u, 1]
    centered per key block x host-precomputed coefficients [s, -2 s t, s t^2]),
    then a single ScalarEngine exp(-x) pass yields attention weights in bf16.
  * attn @ value accumulates out^T (65 x M) in PSUM with the value tile as the
    stationary operand; a ones-column in the value tile produces the softmax
    denominator for free.  Normalization uses a K=1 broadcast matmul + DVE mult.
  * The sorted->natural row un-permutation runs on the otherwise-idle GPSIMD
    engine via local_scatter on (head-pair x M) bf16 tiles.
  * Output projection is a bf16 matmul per 128-row block; per-core partials
    (one per batch half) are summed on the host.
"""

import sys
import types

import numpy as np
import ml_dtypes

B, M, HS, NH, D = 2, 2048, 1024, 16, 64
NCORES = 8
HPC = 4            # heads per core
CP = HPC * D       # 256-wide channel slice per core
SUB = 512          # scores i sub-chunk
NJB = M // 128     # key blocks
TAIL_T = 50.0      # window cut: exp(-TAIL_T) ~ 2e-22

_CACHE = {}        # ranges tuple -> compiled Bass program


def _ensure_ntff_hook():
    """Install the antenv.axon_hooks shim if the image lacks it (profiling only)."""
    try:
        import antenv.axon_hooks  # noqa: F401
        return
    except ImportError:
        pass
    try:
        import antenv
        from trn_agent_boot.trn_boot import _ntff_profile_via_ctypes
    except ImportError:
        return
    mod = types.ModuleType("antenv.axon_hooks")
    _h = [None]
    mod.set_axon_ntff_profile_hook = lambda hk: _h.__setitem__(0, hk)
    mod.get_axon_ntff_profile_hook = lambda: _h[0]
    sys.modules["antenv.axon_hooks"] = mod
    antenv.axon_hooks = mod
    try:
        mod.set_axon_ntff_profile_hook(
            _ntff_profile_via_ctypes("/opt/axon/libaxon_pjrt.so"))
    except Exception:
        pass


def _sigmoid64(x):
    return 1.0 / (1.0 + np.exp(-x.astype(np.float64)))


def _softplus64(x):
    return np.logaddexp(0.0, x.astype(np.float64))


def _build_host_data(h, W_span, W_val, W_out):
    h = np.asarray(h, np.float32)
    W_span = np.asarray(W_span, np.float32)
    W_val = np.asarray(W_val, np.float32)
    W_out = np.asarray(W_out, np.float32)

    span = (h.reshape(B * M, HS) @ W_span.T).reshape(B, M, 2 * NH)

    m_all = np.zeros((B, NH, M), np.float64)
    s_all = np.zeros((B, NH, M), np.float64)
    for b in range(B):
        for k in range(NH):
            m_all[b, k] = _sigmoid64(span[b, :, 2 * k]) * M
            s_all[b, k] = _softplus64(span[b, :, 2 * k + 1])
    order_all = np.argsort(m_all, axis=-1, kind="stable")
    W_all = np.sqrt(TAIL_T / np.maximum(s_all, 1e-12))

    ilos = np.full(NJB, M, np.int64)
    ihis = np.zeros(NJB, np.int64)
    for b in range(B):
        for k in range(NH):
            ms = m_all[b, k][order_all[b, k]]
            ws = W_all[b, k][order_all[b, k]]
            lo, hi = ms - ws, ms + ws
            for jb in range(NJB):
                mask = (hi >= jb * 128) & (lo <= jb * 128 + 128)
                idx = np.flatnonzero(mask)
                if idx.size:
                    ilos[jb] = min(ilos[jb], idx[0])
                    ihis[jb] = max(ihis[jb], idx[-1] + 1)
    ranges = []
    for jb in range(NJB):
        if ihis[jb] <= ilos[jb]:
            ranges.append((0, 0))
        else:
            ranges.append((int(ilos[jb]) & ~7, min(M, (int(ihis[jb]) + 7) & ~7)))

    # coverage: every sorted row must fall in the range of its own mean's block
    for b in range(B):
        for k in range(NH):
            ms = m_all[b, k][order_all[b, k]]
            own = np.clip((ms // 128).astype(np.int64), 0, NJB - 1)
            pos = np.arange(M)
            lows = np.array([ranges[j][0] for j in own])
            highs = np.array([ranges[j][1] for j in own])
            if not ((lows <= pos) & (pos < highs)).all():
                raise AssertionError("window schedule does not cover all rows")

    offs, cw = [], 0
    for lo, hi in ranges:
        offs.append(cw)
        cw += hi - lo

    in_maps = []
    for core in range(NCORES):
        b, g = core // HPC, core % HPC
        heads = [g * HPC + kk for kk in range(HPC)]

        hTb = np.ascontiguousarray(h[b].T).astype(ml_dtypes.bfloat16)
        Wv = np.ascontiguousarray(W_val[g * CP:(g + 1) * CP, :].T).astype(ml_dtypes.bfloat16)
        Wo = np.ascontiguousarray(W_out[:, g * CP:(g + 1) * CP].T).astype(ml_dtypes.bfloat16)

        A3 = np.zeros((HPC, 3, cw), np.float32)
        sidx = np.zeros((2, 2, 128, M), np.int16)
        for kk, k in enumerate(heads):
            order = order_all[b, k]
            ms = m_all[b, k][order]
            ss = s_all[b, k][order]
            for jb in range(NJB):
                lo, hi = ranges[jb]
                if hi <= lo:
                    continue
                t = ms[lo:hi] - (128.0 * jb + 64.0)
                s_ = ss[lo:hi]
                o = offs[jb]
                A3[kk, 0, o:o + hi - lo] = s_
                A3[kk, 1, o:o + hi - lo] = -2.0 * s_ * t
                A3[kk, 2, o:o + hi - lo] = s_ * t * t
            pair, sub = kk // 2, kk % 2
            o64 = order.astype(np.int64)
            iA = np.where(o64 < M // 2, o64, -1).astype(np.int16)
            iB = np.where(o64 >= M // 2, o64 - M // 2, -1).astype(np.int16)
            rows = slice(64 * sub, 64 * sub + 64)
            sidx[pair, 0, rows, :] = iA[None, :]
            sidx[pair, 1, rows, :] = iB[None, :]

        u = np.arange(-64, 64, dtype=np.float32)
        basis = np.stack([u * u, u, np.ones(128, np.float32)])

        in_maps.append({
            "hTb": hTb, "Wv": Wv, "Wo": Wo,
            "A3": A3,
            "sidx": sidx, "basis": basis,
        })

    return in_maps, tuple(ranges)


def _build_kernel(ranges):
    import concourse.tile as tile
    from concourse import bacc, mybir
    from concourse.alu_op_type import AluOpType

    F32 = mybir.dt.float32
    BF16 = mybir.dt.bfloat16
    I16 = mybir.dt.int16

    nc = bacc.Bacc("TRN2", target_bir_lowering=False, debug=False, num_devices=NCORES)

    offs, cw = [], 0
    for lo, hi in ranges:
        offs.append(cw)
        cw += hi - lo
    hTb = nc.dram_tensor("hTb", [HS, M], BF16, kind="ExternalInput")
    Wv = nc.dram_tensor("Wv", [HS, CP], BF16, kind="ExternalInput")
    Wo = nc.dram_tensor("Wo", [CP, HS], BF16, kind="ExternalInput")
    A3 = nc.dram_tensor("A3", [HPC, 3, cw], F32, kind="ExternalInput")
    sidx = nc.dram_tensor("sidx", [2, 2, 128, M], I16, kind="ExternalInput")
    basis = nc.dram_tensor("basis", [3, 128], F32, kind="ExternalInput")
    out_part = nc.dram_tensor("out_part", [M, HS], BF16, kind="ExternalOutput")

    NC8 = HS // 128
    copy_ctr = [0]

    def copy_cast(out_ap, in_ap):
        if copy_ctr[0] % 2 == 0:
            nc.vector.tensor_copy(out_ap, in_ap)
        else:
            nc.scalar.copy(out_ap, in_ap)
        copy_ctr[0] += 1

    with tile.TileContext(nc) as tc:
        with (
            tc.tile_pool(name="persist", bufs=1) as persist,
            tc.tile_pool(name="vpool", bufs=1) as vpool,
            tc.tile_pool(name="attn_pool", bufs=3) as attn_pool,
            tc.tile_pool(name="norm_pool", bufs=4) as norm_pool,
            tc.tile_pool(name="out_pool", bufs=3) as out_pool,
            tc.tile_pool(name="ps", bufs=2, space="PSUM") as ps,
        ):
            hT_sb = []
            for c in range(NC8):
                t = persist.tile([128, M], BF16, name=f"hT{c}")
                nc.sync.dma_start(t[:], hTb[c * 128:(c + 1) * 128, :])
                hT_sb.append(t)
            Wv_sb = []
            for c in range(NC8):
                t = persist.tile([128, CP], BF16, name=f"Wv{c}")
                nc.sync.dma_start(t[:], Wv[c * 128:(c + 1) * 128, :])
                Wv_sb.append(t)
            Wo_sb = []
            for p in range(2):
                t = persist.tile([128, HS], BF16, name=f"Wo{p}")
                nc.sync.dma_start(t[:], Wo[p * 128:(p + 1) * 128, :])
                Wo_sb.append(t)
            A_sb = []
            for kk in range(HPC):
                t = persist.tile([3, cw], F32, name=f"A{kk}")
                nc.sync.dma_start(t[:], A3[kk])
                A_sb.append(t)
            sidx_sb = []
            for p in range(2):
                row = []
                for hh in range(2):
                    t = persist.tile([128, M], I16, name=f"sidx{p}{hh}")
                    nc.sync.dma_start(t[:], sidx[p, hh])
                    row.append(t)
                sidx_sb.append(row)
            basis_sb = persist.tile([3, 128], F32, name="basis")
            nc.sync.dma_start(basis_sb[:], basis[:])
            ones_sb = persist.tile([1, 64], F32, name="ones64")
            nc.vector.memset(ones_sb[:], 1.0)

            pair_sb = [persist.tile([128, M], BF16, name=f"pair{p}") for p in range(2)]
            nat_sb = [persist.tile([128, M], BF16, name=f"nat{p}") for p in range(2)]

            # ---- value (per key block), with a ones column per head ----
            v_sb = []
            for jb in range(NJB):
                vt = vpool.tile([128, HPC * (D + 1)], BF16, name=f"v{jb}")
                pv = ps.tile([128, CP], F32, name="pv", tag="sc", bufs=2)
                for c in range(NC8):
                    nc.tensor.matmul(
                        pv[:], hT_sb[c][:, jb * 128:(jb + 1) * 128], Wv_sb[c][:],
                        start=(c == 0), stop=(c == NC8 - 1))
                for kk in range(HPC):
                    copy_cast(vt[:, kk * (D + 1):kk * (D + 1) + D],
                              pv[:, kk * D:(kk + 1) * D])
                    nc.vector.memset(vt[:, kk * (D + 1) + D:kk * (D + 1) + D + 1], 1.0)
                v_sb.append(vt)

            # ---- scores + attention per head ----
            for kk in range(HPC):
                pair, sub = kk // 2, kk % 2
                o_ps = ps.tile([65, M], F32, name="oT", tag="oT", bufs=1)
                nc.scalar.memzero(o_ps[:])
                for jb in range(NJB):
                    lo, hi = ranges[jb]
                    if hi <= lo:
                        continue
                    for s0 in range(lo, hi, SUB):
                        s1 = min(hi, s0 + SUB)
                        w = s1 - s0
                        sc = ps.tile([128, SUB], F32, name="sc", tag="sc", bufs=2)
                        nc.tensor.matmul(
                            sc[:, :w], basis_sb[:],
                            A_sb[kk][:, offs[jb] + s0 - lo: offs[jb] + s1 - lo],
                            start=True, stop=True)
                        at = attn_pool.tile([128, SUB], BF16, name="at", tag="at")
                        nc.scalar.activation(at[:, :w], sc[:, :w],
                                             mybir.ActivationFunctionType.Exp, scale=-1.0)
                        nc.tensor.matmul(
                            o_ps[:, s0:s1],
                            v_sb[jb][:, kk * (D + 1):(kk + 1) * (D + 1)],
                            at[:, :w], start=False, stop=False)
                for q in range(M // SUB):
                    qs = slice(q * SUB, (q + 1) * SUB)
                    rcr = norm_pool.tile([1, SUB], F32, name="rcr", tag="rcr")
                    nc.scalar.copy(rcr[:], o_ps[64:65, qs])
                    rc = norm_pool.tile([1, SUB], F32, name="rc", tag="rc")
                    nc.vector.reciprocal_approx_fast(rc[:], rcr[:])
                    bc = ps.tile([64, SUB], F32, name="bc", tag="sc", bufs=2)
                    nc.tensor.matmul(bc[:], ones_sb[:], rc[:], start=True, stop=True)
                    bcs = norm_pool.tile([64, SUB], F32, name="bcs", tag="bcs")
                    nc.scalar.copy(bcs[:], bc[:])
                    nc.vector.tensor_tensor(
                        pair_sb[pair][64 * sub:64 * sub + 64, qs],
                        o_ps[0:64, qs], bcs[:], AluOpType.mult)

            # ---- un-permute sorted -> natural (gpsimd) ----
            for p in range(2):
                for hh in range(2):
                    nc.gpsimd.local_scatter(
                        nat_sb[p][:, hh * (M // 2):(hh + 1) * (M // 2)],
                        pair_sb[p][:], sidx_sb[p][hh][:],
                        channels=128, num_elems=M // 2, num_idxs=M)

            # ---- output projection ----
            for ic in range(M // 128):
                ics = slice(ic * 128, (ic + 1) * 128)
                ot = out_pool.tile([128, HS], BF16, name="ot", tag="ot")
                for jh in range(2):
                    jhs = slice(jh * 512, (jh + 1) * 512)
                    pp = ps.tile([128, 512], F32, name="pp", tag="pp", bufs=2)
                    nc.tensor.matmul(pp[:], nat_sb[0][:, ics], Wo_sb[0][:, jhs],
                                     start=True, stop=False)
                    nc.tensor.matmul(pp[:], nat_sb[1][:, ics], Wo_sb[1][:, jhs],
                                     start=False, stop=True)
                    copy_cast(ot[:, jhs], pp[:])
                nc.sync.dma_start(out_part[ics, :], ot[:])

    nc.compile()
    return nc


def kernel(h, W_span, W_val, W_out):
    _ensure_ntff_hook()
    from concourse.bass_utils import run_bass_kernel_spmd

    in_maps, ranges = _build_host_data(h, W_span, W_val, W_out)
    nc = _CACHE.get(ranges)
    if nc is None:
        nc = _build_kernel(ranges)
        _CACHE[ranges] = nc

    res = run_bass_kernel_spmd(nc, in_maps, list(range(NCORES)), trace=False)

    out = np.zeros((B, M, HS), np.float32)
    for core in range(NCORES):
        out[core // HPC] += res.results[core]["out_part"].astype(np.float32)
    return out
